# revision 8
# baseline (speedup 1.0000x reference)
"""Coordinate-Attention kernel for Trainium2, data-parallel over batch on 8 NeuronCores.

Reference computation (per batch b):
  xh[c,h] = mean_w x[c,h,w]; xw[c,w] = mean_h x[c,h,w]
  y = hswish(BN(w1 @ concat(xh, xw) + b1))            # [Cm=128, 128]
  gh = sigmoid(w2 @ y[:, :64] + b2)                    # [256, 64]
  gw = sigmoid(w3 @ y[:, 64:] + b3)                    # [256, 64]
  out[c,h,w] = x[c,h,w] * gh[c,h] * gw[c,w]

Host folds BN into w1/b1 and the 1/64 pooling mean into w1. Each core
processes 4 batches; x is sharded on B across the 8 cores.
"""
import sys

for _p in ("/opt/trn_rl_repo",):
    if _p not in sys.path:
        sys.path.insert(0, _p)

import numpy as np

import concourse.bacc as bacc
import concourse.bass as bass
import concourse.tile as tile
import concourse.mybir as mybir
from concourse.bass_utils import run_bass_kernel_spmd

N_CORES = 8
B, C, H, W = 32, 256, 64, 64
B_LOC = B // N_CORES  # 4
CB = C // 128  # 2 channel blocks
F32 = mybir.dt.float32
AF = mybir.ActivationFunctionType
ALU = mybir.AluOpType
AX = mybir.AxisListType

_NC_CACHE = {}


def build_module(n_iter: int = 1):
    """Build + compile the Bacc module. n_iter>1 wraps the whole workload in a
    hardware For_i loop (used only for timing; the graded path uses n_iter=1)."""
    nc = bacc.Bacc("TRN2", debug=False, num_devices=N_CORES)
    x_d = nc.dram_tensor("x", [B_LOC, C, H, W], F32, kind="ExternalInput").ap()
    w1t_d = nc.dram_tensor("w1t", [128, CB, 128], F32, kind="ExternalInput").ap()
    b1c_d = nc.dram_tensor("b1c", [128, 1], F32, kind="ExternalInput").ap()
    w2t_d = nc.dram_tensor("w2t", [128, 2, 128], F32, kind="ExternalInput").ap()
    b2c_d = nc.dram_tensor("b2c", [128, 2], F32, kind="ExternalInput").ap()
    w3t_d = nc.dram_tensor("w3t", [128, 2, 128], F32, kind="ExternalInput").ap()
    b3c_d = nc.dram_tensor("b3c", [128, 2], F32, kind="ExternalInput").ap()
    out_d = nc.dram_tensor("out", [B_LOC, C, H, W], F32, kind="ExternalOutput").ap()

    from contextlib import ExitStack

    with tile.TileContext(nc) as tc, ExitStack() as ctx:
        singles = ctx.enter_context(tc.tile_pool(name="singles", bufs=1))
        xs_pool = ctx.enter_context(tc.tile_pool(name="xs", bufs=2))
        out_pool = ctx.enter_context(tc.tile_pool(name="outs", bufs=2))
        med_pool = ctx.enter_context(tc.tile_pool(name="med", bufs=2))
        small_pool = ctx.enter_context(tc.tile_pool(name="small", bufs=3))
        zp_pool = ctx.enter_context(tc.tile_pool(name="zp", bufs=2, space="PSUM"))
        gp_pool = ctx.enter_context(tc.tile_pool(name="gp", bufs=2, space="PSUM"))

        def load_weights():
            w1t_sb = singles.tile([128, CB, 128], F32, name="w1t_sb", tag="w1t_sb")
            nc.sync.dma_start(out=w1t_sb, in_=w1t_d)
            b1c_sb = singles.tile([128, 1], F32, name="b1c_sb", tag="b1c_sb")
            nc.sync.dma_start(out=b1c_sb, in_=b1c_d)
            w2t_sb = singles.tile([128, 2, 128], F32, name="w2t_sb", tag="w2t_sb")
            nc.sync.dma_start(out=w2t_sb, in_=w2t_d)
            b2c_sb = singles.tile([128, 2], F32, name="b2c_sb", tag="b2c_sb")
            nc.sync.dma_start(out=b2c_sb, in_=b2c_d)
            w3t_sb = singles.tile([128, 2, 128], F32, name="w3t_sb", tag="w3t_sb")
            nc.sync.dma_start(out=w3t_sb, in_=w3t_d)
            b3c_sb = singles.tile([128, 2], F32, name="b3c_sb", tag="b3c_sb")
            nc.sync.dma_start(out=b3c_sb, in_=b3c_d)
            half_sb = singles.tile([128, 1], F32, name="half_sb", tag="half_sb")
            nc.vector.memset(half_sb, 0.5)
            return w1t_sb, b1c_sb, w2t_sb, b2c_sb, w3t_sb, b3c_sb, half_sb

        def body(weights):
            w1t_sb, b1c_sb, w2t_sb, b2c_sb, w3t_sb, b3c_sb, half_sb = weights
            for b in range(B_LOC):
                xt = [None, None]
                pooled = med_pool.tile([128, CB * 128], F32, name="pooled", tag="pooled")
                for cb in range(CB):
                    xt[cb] = xs_pool.tile(
                        [128, H, W], F32, name=f"xt{cb}", tag=f"xt{cb}"
                    )
                    nc.sync.dma_start(
                        out=xt[cb], in_=x_d[b, cb * 128 : (cb + 1) * 128]
                    )
                    # pool over W -> [c, h] ; pool over H -> [c, w]
                    nc.vector.reduce_sum(
                        out=pooled[:, cb * 128 : cb * 128 + 64], in_=xt[cb], axis=AX.X
                    )
                    nc.vector.reduce_sum(
                        out=pooled[:, cb * 128 + 64 : cb * 128 + 128],
                        in_=xt[cb].transpose([0, 2, 1]),
                        axis=AX.X,
                    )
                # z = w1' @ pooled  (accumulate over the two c blocks)
                zp = zp_pool.tile([128, 128], F32, name="zp", tag="zp")
                for cb in range(CB):
                    nc.tensor.matmul(
                        zp,
                        lhsT=w1t_sb[:, cb, :],
                        rhs=pooled[:, cb * 128 : (cb + 1) * 128],
                        start=(cb == 0),
                        stop=(cb == CB - 1),
                    )
                # y = hswish(z + b1') = s * clip(s/6 + 0.5, 0, 1) with s = z + b1'
                s_t = small_pool.tile([128, 128], F32, name="s_t", tag="s_t")
                nc.vector.tensor_scalar_add(s_t, zp, b1c_sb[:, 0:1])
                t_t = small_pool.tile([128, 128], F32, name="t_t", tag="t_t")
                nc.scalar.activation(
                    t_t, s_t, AF.Relu, bias=half_sb[:, 0:1], scale=1.0 / 6.0
                )
                nc.vector.tensor_scalar_min(t_t, t_t, 1.0)
                y_t = small_pool.tile([128, 128], F32, name="y_t", tag="y_t")
                nc.vector.tensor_mul(y_t, s_t, t_t)
                # gates
                gh_t = small_pool.tile([128, 2, 64], F32, name="gh_t", tag="gh_t")
                gw_t = small_pool.tile([128, 2, 64], F32, name="gw_t", tag="gw_t")
                for ob in range(2):
                    ghp = gp_pool.tile([128, 64], F32, name="ghp", tag="ghp")
                    nc.tensor.matmul(
                        ghp, lhsT=w2t_sb[:, ob, :], rhs=y_t[:, 0:64],
                        start=True, stop=True,
                    )
                    nc.scalar.activation(
                        gh_t[:, ob, :], ghp, AF.Sigmoid, bias=b2c_sb[:, ob : ob + 1]
                    )
                    gwp = gp_pool.tile([128, 64], F32, name="gwp", tag="gwp")
                    nc.tensor.matmul(
                        gwp, lhsT=w3t_sb[:, ob, :], rhs=y_t[:, 64:128],
                        start=True, stop=True,
                    )
                    nc.scalar.activation(
                        gw_t[:, ob, :], gwp, AF.Sigmoid, bias=b3c_sb[:, ob : ob + 1]
                    )
                # out = x * gh (bcast over w) * gw (bcast over h)
                for cb in range(CB):
                    ot = out_pool.tile([128, H, W], F32, name=f"ot{cb}", tag=f"ot{cb}")
                    gw_b = gw_t[:, cb, :].unsqueeze(1).broadcast_to([128, H, W])
                    gh_b = gh_t[:, cb, :].unsqueeze(2).broadcast_to([128, H, W])
                    nc.vector.tensor_mul(ot, xt[cb], gw_b)
                    nc.vector.tensor_mul(ot, ot, gh_b)
                    nc.sync.dma_start(
                        out=out_d[b, cb * 128 : (cb + 1) * 128], in_=ot
                    )

        if n_iter == 1:
            body(load_weights())
        else:
            with tc.For_i(0, n_iter, 1):
                body(load_weights())
    nc.compile()
    return nc


def get_module(n_iter: int = 1):
    if n_iter not in _NC_CACHE:
        _NC_CACHE[n_iter] = build_module(n_iter)
    return _NC_CACHE[n_iter]


def make_in_maps(x, w1, b1, bn_gamma, bn_beta, bn_mean, bn_var, w2, b2, w3, b3):
    f64 = np.float64
    s_bn = (bn_gamma.astype(f64) / np.sqrt(bn_var.astype(f64) + 1e-5))
    w1p = (w1.astype(f64) * s_bn[:, None] / 64.0).astype(np.float32)  # [128, 256]
    b1c = ((b1.astype(f64) - bn_mean.astype(f64)) * s_bn + bn_beta.astype(f64)).astype(
        np.float32
    )
    consts = {
        "w1t": np.ascontiguousarray(w1p.T.reshape(CB, 128, 128).transpose(1, 0, 2)),
        "b1c": np.ascontiguousarray(b1c.reshape(128, 1)),
        "w2t": np.ascontiguousarray(w2.T.reshape(128, 2, 128)),
        "b2c": np.ascontiguousarray(b2.reshape(2, 128).T),
        "w3t": np.ascontiguousarray(w3.T.reshape(128, 2, 128)),
        "b3c": np.ascontiguousarray(b3.reshape(2, 128).T),
    }
    x = np.ascontiguousarray(x, dtype=np.float32)
    in_maps = []
    for i in range(N_CORES):
        m = {"x": x[i * B_LOC : (i + 1) * B_LOC]}
        m.update(consts)
        in_maps.append(m)
    return in_maps


def kernel(**inputs) -> np.ndarray:
    nc = get_module(1)
    in_maps = make_in_maps(**inputs)
    res = run_bass_kernel_spmd(nc, in_maps, core_ids=list(range(N_CORES)))
    out = np.concatenate([res.results[i]["out"] for i in range(N_CORES)], axis=0)
    return out.astype(np.float32, copy=False)
